# revision 9
# baseline (speedup 1.0000x reference)
"""Trainium2 Bass kernel for nn_Process_new_61624190763140.

Math background
---------------
The reference integrates, per sample b, k_b steps (k_b = iterations_for_sample[b],
0..99) of the map

    T(x) = x + h * F(x),    F(x) = D^T r(x),  r = exp(S log x),  D = prod - sub,

(h = 1e-4; within one step the reference applies the 20 reactions sequentially,
but replacing that by the parallel "Jacobi" update costs only ~4e-6 relative
error -- measured), followed by a correlation matrix over the batch.

Because every sample applies the *same* map, T^{k} admits a 2nd-order Taylor
composition in h with *per-sample* coefficients:

    T^k(x) ~= x + c1 * h F(x) + c2 * h^2 (JF)F(x),
    c1 = k,  c2 = k(k-1)/2,
    (JF)F = D^T [ r * S((F/x)) ]        (the 1/x poles cancel against r)

With n*h <= 1e-2 the truncation error is ~1e-7, far below the 4e-6 Jacobi
floor (verified numerically: rel_x = 4.0e-6, rel_c = 1.1e-6 vs the reference).
So the whole 100-iteration scan collapses into ~15 engine instructions.

Sharding strategy: the problem is latency-bound (70KB total I/O); any
cross-core collective costs ~10us which exceeds the whole compute. So every
core redundantly computes the full batch (replicated data-parallel) and we
read core 0's output. All 8 cores run the same SPMD program.

Layout: x^T [20 metabolites (partitions), 400 samples (free)]. All matmuls
contract over the 20-partition dim (K=20). The correlation needs sample-major
chunks, produced with 4 PE transposes.
"""

import numpy as np

M = 20       # metabolites
R = 20       # reactions
B = 400      # batch
H = 1e-4     # step
NCORES = 8
CHUNK = 100  # transpose chunk width (4 chunks of 100 samples)

_CACHE = {}


def _to_np(a, dtype=None):
    a = np.asarray(a)
    if dtype is not None:
        a = a.astype(dtype)
    return a


def build_nc():
    """Build + compile the Bass program; returns the compiled Bacc."""
    import concourse.bass as bass
    import concourse.bacc as bacc
    import concourse.mybir as mybir
    import concourse.tile as tile

    f32 = mybir.dt.float32
    AF = mybir.ActivationFunctionType
    ALU = mybir.AluOpType
    AX = mybir.AxisListType

    nc = bacc.Bacc(
        "TRN2",
        target_bir_lowering=False,
        debug=False,
        enable_asserts=True,
        num_devices=NCORES,
    )

    # Packed input: [20, 1261] = xT(400) | c1b(400) | c2b(400) | ST(20) | Ds(20) | eye(20) | eps(1)
    NIN = B * 3 + M * 3 + 1
    inp = nc.dram_tensor("inp", [M, NIN], f32, kind="ExternalInput").ap()
    # Packed output: [20, 420] = xfT(400) | C(20)
    out = nc.dram_tensor("out", [M, B + M], f32, kind="ExternalOutput").ap()

    with tile.TileContext(nc) as tc:
        with (
            tc.tile_pool(name="sb", bufs=1) as sb,
            tc.tile_pool(name="ps", bufs=2, space="PSUM") as ps,
            tc.tile_pool(name="pst", bufs=2, space="PSUM") as pst,
            tc.tile_pool(name="psc", bufs=1, space="PSUM") as psc,
        ):
            sin = sb.tile([M, NIN], f32, tag="sin")
            nc.sync.dma_start(sin[:], inp[:])
            xT = sin[:, 0:B]
            c1b = sin[:, B:2 * B]
            c2b = sin[:, 2 * B:3 * B]
            ST = sin[:, 3 * B:3 * B + M]          # S^T  (lhsT for g = S @ l)
            Ds = sin[:, 3 * B + M:3 * B + 2 * M]  # h*D  (lhsT for F' = h D^T r)
            eye = sin[:, 3 * B + 2 * M:3 * B + 3 * M]
            eps = sin[:, 3 * B + 3 * M:3 * B + 3 * M + 1]

            # --- mega-step ---
            ell = sb.tile([M, B], f32, tag="ell")
            nc.scalar.activation(ell[:], xT, AF.Ln, bias=eps)

            g = ps.tile([M, B], f32, tag="mm")
            nc.tensor.matmul(g[:], ST, ell[:], start=True, stop=True)

            r = sb.tile([M, B], f32, tag="r")
            nc.scalar.activation(r[:], g[:], AF.Exp)

            xinv = sb.tile([M, B], f32, tag="xinv")   # 1/x = exp(-log x)
            nc.scalar.activation(xinv[:], ell[:], AF.Exp, scale=-1.0)

            Fp = ps.tile([M, B], f32, tag="mm")       # h*F
            nc.tensor.matmul(Fp[:], Ds, r[:], start=True, stop=True)

            v = sb.tile([M, B], f32, tag="v")         # h*F/x
            nc.vector.tensor_mul(v[:], Fp[:], xinv[:])

            w = ps.tile([M, B], f32, tag="mm")        # S v
            nc.tensor.matmul(w[:], ST, v[:], start=True, stop=True)

            z = sb.tile([M, B], f32, tag="z")         # r * Sv
            nc.vector.tensor_mul(z[:], r[:], w[:])

            rc1 = sb.tile([M, B], f32, tag="rc1")     # r * c1   (off critical path)
            nc.gpsimd.tensor_mul(rc1[:], r[:], c1b)

            q = sb.tile([M, B], f32, tag="q")         # z*c2 + r*c1
            nc.vector.tensor_mul(q[:], z[:], c2b)
            nc.vector.tensor_add(q[:], q[:], rc1[:])

            U = ps.tile([M, B], f32, tag="mm")        # h D^T q  = c1 hF + c2 h^2 (JF)F
            nc.tensor.matmul(U[:], Ds, q[:], start=True, stop=True)

            xf = sb.tile([M, B], f32, tag="xf")
            nc.vector.tensor_add(xf[:], xT, U[:])
            nc.sync.dma_start(out[:, 0:B], xf[:])

            # --- correlation matrix ---
            # mean via ACT accumulator: out junk = xf/B, accum = sum = mean
            xscaled = sb.tile([M, B], f32, tag="xscaled")
            mean = sb.tile([M, 1], f32, tag="mean")
            nc.scalar.activation(xscaled[:], xf[:], AF.Copy, scale=1.0 / B,
                                 accum_out=mean[:])

            vb = sb.tile([M, B], f32, tag="vb")
            nc.vector.tensor_scalar_sub(vb[:], xf[:], mean[:])

            sq = sb.tile([M, B], f32, tag="sq")
            var = sb.tile([M, 1], f32, tag="var")
            nc.scalar.activation(sq[:], vb[:], AF.Square, accum_out=var[:])

            lnv = sb.tile([M, 1], f32, tag="lnv")
            nc.scalar.activation(lnv[:], var[:], AF.Ln)
            mr = sb.tile([M, 1], f32, tag="mr")       # var^-0.5 = exp(-0.5 ln var)
            nc.scalar.activation(mr[:], lnv[:], AF.Exp, scale=-0.5)

            vh = sb.tile([M, B], f32, tag="vh")       # vb * mr  (row scaled)
            nc.vector.tensor_scalar_mul(vh[:], vb[:], mr[:])

            # C = sum_chunks  vh_chunk^T-major matmuls
            nch = B // CHUNK
            trs = []
            for c in range(nch):
                trp = pst.tile([CHUNK, M], f32, tag="trp")
                nc.tensor.transpose(trp[:], vh[:, c * CHUNK:(c + 1) * CHUNK], eye)
                t = sb.tile([CHUNK, M], f32, tag=f"trs{c % 2}")
                nc.scalar.copy(t[:], trp[:])
                trs.append(t)
            Cp = psc.tile([M, M], f32, tag="cps")
            for c in range(nch):
                nc.tensor.matmul(Cp[:], trs[c][:], trs[c][:],
                                 start=(c == 0), stop=(c == nch - 1))
            Cs = sb.tile([M, M], f32, tag="cs")
            nc.scalar.copy(Cs[:], Cp[:])
            nc.sync.dma_start(out[:, B:B + M], Cs[:])

    nc.compile()
    return nc


def _build():
    from concourse.bass_utils import run_bass_kernel_spmd

    nc = build_nc()

    def run(packed, trace=False, **kw):
        in_maps = [{"inp": packed} for _ in range(NCORES)]
        res = run_bass_kernel_spmd(nc, in_maps, core_ids=list(range(NCORES)),
                                   trace=trace, **kw)
        return res

    return run


def _pack_inputs(x, sub, prod, iterations_for_sample):
    x = _to_np(x, np.float32)
    sub = _to_np(sub, np.float32)
    prod = _to_np(prod, np.float32)
    k = _to_np(iterations_for_sample).astype(np.float64)

    c1 = k
    c2 = k * (k - 1.0) / 2.0
    packed = np.empty((M, B * 3 + M * 3 + 1), np.float32)
    packed[:, 0:B] = x.T
    packed[:, B:2 * B] = np.broadcast_to(c1.astype(np.float32), (M, B))
    packed[:, 2 * B:3 * B] = np.broadcast_to(c2.astype(np.float32), (M, B))
    packed[:, 3 * B:3 * B + M] = sub.T                      # S^T
    packed[:, 3 * B + M:3 * B + 2 * M] = np.float32(H) * (prod - sub)  # h*D (row-major [r, m])
    packed[:, 3 * B + 2 * M:3 * B + 3 * M] = np.eye(M, dtype=np.float32)
    packed[:, 3 * B + 3 * M] = np.float32(1e-38)
    return packed


def _unpack(out_packed, iterations_dtype):
    xf = np.ascontiguousarray(out_packed[:, 0:B].T).astype(np.float32)
    C = np.ascontiguousarray(out_packed[:, B:B + M]).astype(np.float32)
    return xf, C


def kernel(x, sub, prod, iterations_for_sample):
    if "run" not in _CACHE:
        _CACHE["run"] = _build()
    packed = _pack_inputs(x, sub, prod, iterations_for_sample)
    res = _CACHE["run"](packed)
    out_packed = res.results[0]["out"]
    xf, C = _unpack(out_packed, np.asarray(iterations_for_sample).dtype)
    return xf, C


def _ensure_ntff_hook():
    """The agent image's antenv lacks axon_hooks; synthesize it so
    run_bass_kernel_spmd(trace=True) can capture NTFF profiles."""
    try:
        from antenv.axon_hooks import get_axon_ntff_profile_hook  # noqa: F401
        return
    except ImportError:
        pass
    import sys
    import types

    import antenv
    from trn_agent_boot.trn_boot import _ntff_profile_via_ctypes

    mod = types.ModuleType("antenv.axon_hooks")
    state = {"hook": None}
    mod.set_axon_ntff_profile_hook = lambda h: state.__setitem__("hook", h)
    mod.get_axon_ntff_profile_hook = lambda: state["hook"]
    sys.modules["antenv.axon_hooks"] = mod
    antenv.axon_hooks = mod
    mod.set_axon_ntff_profile_hook(
        _ntff_profile_via_ctypes("/opt/axon/libaxon_pjrt.so")
    )

    # upload_artifacts pushes to fish/S3 which this container can't reach;
    # keep the artifacts local instead.
    import concourse.bass_utils as bu

    bu.upload_artifacts = lambda tmpdir: str(tmpdir)


def kernel_traced(x, sub, prod, iterations_for_sample, **kw):
    """Like kernel() but returns (outputs, BassKernelResults) with trace."""
    _ensure_ntff_hook()
    if "run" not in _CACHE:
        _CACHE["run"] = _build()
    packed = _pack_inputs(x, sub, prod, iterations_for_sample)
    res = _CACHE["run"](packed, trace=True, **kw)
    out_packed = res.results[0]["out"]
    xf, C = _unpack(out_packed, np.asarray(iterations_for_sample).dtype)
    return (xf, C), res
